# revision 13
# baseline (speedup 1.0000x reference)
"""Trainium2 Bass kernel: block 8x8 2D-DCT + channel-pack + 8x nearest upsample.

Computes, for input x (8, 3, 256, 256) f32:
  out[b, 64c+8a+d, 8i+r, 8j+q] = sum_{m,n} D[a,m] x[b,c,8i+m,8j+n] D[d,n]
i.e. the reference nn_DCT2D: per-8x8-block orthonormal DCT-II, 64 coeffs packed
into channels, then 8x8 nearest-neighbor upsample back to (256, 256).

Strategy (pure data-parallel over batch, one core per batch element):
  - Consts: one [128, 640] f32 tensor (M''[kt=0] | M''[kt=1] | R2) loaded in
    a single 128x2.5KB-descriptor HWDGE DMA at t=0 (fast startup).
  - Step 1 (TensorE, f32): A2 = X^T @ M'', the row-DCT over H, where M'' is
    the block-diagonal DCT factor with columns permuted to c'' = ie*128+8*ip+a
    (i = 2*ip + ie). Output A2[kh] [128 x 256] for the two n_img halves.
  - Step 2 (TensorE, f32): per (c, ie, kh) ONE matmul
    ps2[:, kh*128:+128] = A2[kh][:, ie-half]^T @ R2 with R2[k', 16d+jj] =
    D[d, k'%8]*[k'//8 == jj]: all 8 output-channel phases d in one
    [contract 128 x free 128] matmul (no upsample in the matmul).
  - Copy stage A (DVE/ACT): per (c, ie, d) cast ps2[:, 16d:16d+16] f32 ->
    f16 PAIRS qq[p, kh, jj, 2] (64 el) -- each coefficient duplicated twice
    in adjacent f16 slots.
  - Copy stage B (DVE/ACT, 4:3 split favoring DVE): per (c, d, ie, kh)
    broadcast-copy qq bitcast as f32 (one 4-byte element = 2 identical f16)
    to [128, 8r, 16jj, 4qp]: the 8x H-replication and the remaining 4x of
    the W-replication at HALF the element count of a plain f16 fanout.
    Lands in per-(c,d) f16 tiles o4 [128 x 4096], partition p = 8*ip+a,
    free f = ie*2048 + r*256 + kh*128 + jj*8 + q.
  - DMA out (both HWDGE rings, alternating): one 1 MB f16 DMA per (c, d)
    with 8 KB descriptors: partition (ip, a) -> channel 64c+8a+d rows
    [16ip, 16ip+16), contiguous in HBM. The last two tiles are split into
    ie-halves across both rings to shorten the drain tail. Host upcasts
    f16 -> f32 after the gather; f16 storage error (~3e-4 of scale) is far
    inside the 2e-2 gate and halves the dominant HBM write traffic.

Matmuls and PSUM accumulation in f32; only the HBM output tensor is f16.
"""

import numpy as np

import concourse.bacc as bacc
import concourse.mybir as mybir
from concourse.tile import TileContext
from concourse.bass_utils import run_bass_kernel_spmd

N_CORES = 8
B, C, H, W = 8, 3, 256, 256
BS = 8          # DCT block size
F32 = mybir.dt.float32
F16 = mybir.dt.float16


def _dct_matrix() -> np.ndarray:
    n = np.arange(BS, dtype=np.float64)
    k = n[:, None]
    D = np.cos(np.pi * (2.0 * n[None, :] + 1.0) * k / (2.0 * BS))
    scale = np.full((BS,), np.sqrt(2.0 / BS))
    scale[0] = np.sqrt(1.0 / BS)
    return (D * scale[:, None]).astype(np.float32)


def _build_consts() -> np.ndarray:
    D = _dct_matrix()
    # M'' [256, 256]: col c'' = ie*128 + 8*ip + a maps to DCT row
    # 32a + i with i = 2*ip + ie:  M''[k, c''] = D[a, k%8] iff k//8 == i.
    # (ip-major partition order so the output DMA's outer dst dim has 16
    # entries -> descriptors spread over all 16 SDMA engines.)
    Mpp = np.zeros((256, 256), np.float32)
    for k in range(256):
        i = k // 8
        for a in range(8):
            Mpp[k, (i % 2) * 128 + 8 * (i // 2) + a] = D[a, k % 8]

    # R2 [128, 128]: second DCT factor, all d packed in the free dim.
    # R2[k', 16d+jj] = D[d, k'%8] iff k'//8 == jj.
    R2 = np.zeros((128, 128), np.float32)
    for kp in range(128):
        jj = kp // 8
        for d in range(8):
            R2[kp, 16 * d + jj] = D[d, kp % 8]

    # single [128, 640] partition-major tensor: 2.5KB per descriptor.
    return np.concatenate([Mpp[:128], Mpp[128:], R2], axis=1)


def _build_module():
    nc = bacc.Bacc("TRN2", target_bir_lowering=False, debug=False,
                   enable_asserts=False)

    x_t = nc.dram_tensor("x", [C, H, W], F32, kind="ExternalInput")
    c_t = nc.dram_tensor("cst", [128, 640], F32, kind="ExternalInput")
    out_t = nc.dram_tensor("out", [C * 64, H, W], F16, kind="ExternalOutput")
    # view with channel split into (c, a, d) for the output APs
    out_r = out_t.rearrange("(c a d) h w -> c a d h w", c=C, a=8, d=8)

    with TileContext(nc) as tc:
        with (
            tc.tile_pool(name="consts", bufs=1) as cpool,
            tc.tile_pool(name="xp", bufs=6) as xpool,
            tc.tile_pool(name="atp", bufs=4) as atpool,
            tc.tile_pool(name="qqp", bufs=6) as qqpool,
            tc.tile_pool(name="outp", bufs=10) as opool,
            tc.tile_pool(name="psa", bufs=2, space="PSUM") as psa_pool,
            tc.tile_pool(name="ps2", bufs=4, space="PSUM") as ps2_pool,
        ):
            # consts on the Act HWDGE ring, x[c=0] split across both HWDGE
            # rings (all idle at t=0); later x images on the Pool SWDGE ring
            # to keep the HWDGE rings clear for output.
            ct = cpool.tile_from(c_t[:, :], name="consts",
                                 forced_dma_engine=mybir.EngineType.Activation)
            m_ap = [ct[:, 0:256], ct[:, 256:512]]
            r2_ap = ct[:, 512:640]

            xt = []
            for c in range(C):
                for kt in range(2):
                    tile = xpool.tile([128, 256], F32, tag="x")
                    if c == 0:
                        eng = nc.sync if kt == 0 else nc.scalar
                    else:
                        eng = nc.gpsimd
                    eng.dma_start(out=tile[:, :],
                                  in_=x_t[c, kt * 128:(kt + 1) * 128, :])
                    xt.append(tile)

            qi = 0   # output DMA queue round-robin
            cp = 0   # copy engine round-robin
            for c in range(C):
                # step 1: A2[kh] [n_img-half, c''=(ie, ip, a)]
                at = []
                for kh in range(2):
                    ps_a = psa_pool.tile([128, 256], F32, tag="psa")
                    for kt in range(2):
                        nc.tensor.matmul(
                            ps_a[:, :],
                            lhsT=xt[2 * c + kt][:, kh * 128:(kh + 1) * 128],
                            rhs=m_ap[kt],
                            start=(kt == 0), stop=(kt == 1),
                        )
                    a_sb = atpool.tile([128, 256], F32, tag="at")
                    if kh == 0:
                        nc.vector.tensor_copy(out=a_sb[:, :], in_=ps_a[:, :])
                    else:
                        nc.scalar.copy(out=a_sb[:, :], in_=ps_a[:, :])
                    at.append(a_sb)

                # step 2: per (ie, kh) one matmul computing all d at once:
                # ps2[ie][p=(ip,a), kh*128 + 16d + jj] = coeff y[c,(a,d),i,j]
                # with i = 2ip+ie, j = 16kh+jj.
                ps2 = []
                for ie in range(2):
                    t = ps2_pool.tile([128, 256], F32, tag="ps2")
                    for kh in range(2):
                        nc.tensor.matmul(
                            t[:, kh * 128:(kh + 1) * 128],
                            lhsT=at[kh][:, ie * 128:(ie + 1) * 128],
                            rhs=r2_ap,
                            start=True, stop=True,
                        )
                    ps2.append(t)
                ps2v = [p.rearrange("p (kh f) -> p kh f", kh=2) for p in ps2]

                # copies + 1 MB DMA per (c, d)
                for d in range(8):
                    o4 = opool.tile([128, 4096], F16, tag="o4")
                    ov32 = o4.bitcast(F32).rearrange(
                        "p (ie r kh jj qp) -> p ie kh r jj qp",
                        ie=2, r=8, kh=2, jj=16, qp=4)
                    for ie in range(2):
                        # stage A: f16 pairs qq[p, kh, jj, 2]
                        qq = qqpool.tile([128, 64], F16, tag="qq")
                        src = ps2v[ie][:, :, 16 * d:16 * d + 16]
                        src = src[:, :, :, None].to_broadcast([128, 2, 16, 2])
                        dst = qq.rearrange("p (kh jj pr) -> p kh jj pr",
                                           kh=2, jj=16, pr=2)
                        if cp % 2 == 0:
                            nc.vector.tensor_copy(out=dst, in_=src)
                        else:
                            nc.scalar.copy(out=dst, in_=src)
                        cp += 1
                        # stage B: pair-granular fanout, f32 bitcast
                        qq32 = qq.bitcast(F32)
                        for kh in range(2):
                            src32 = qq32[:, kh * 16:(kh + 1) * 16]
                            src32 = src32[:, None, :, None].to_broadcast(
                                [128, 8, 16, 4])
                            if cp % 2 == 0:
                                nc.vector.tensor_copy(out=ov32[:, ie, kh],
                                                      in_=src32)
                            else:
                                nc.scalar.copy(out=ov32[:, ie, kh], in_=src32)
                            cp += 1

                    # one 1 MB f16 DMA: partition (ip, a) -> rows
                    # [16ip, 16ip+16) of channel (c, a, d); 8 KB descriptors.
                    dst = out_r[c, :, d].rearrange(
                        "a (ip hh) w -> ip a (hh w)", hh=16)
                    eng = nc.scalar if qi % 2 == 1 else nc.sync
                    qi += 1
                    eng.dma_start(out=dst, in_=o4[:, :])

    nc.compile()
    return nc


_CACHE: dict = {}


def _get_module():
    if "nc" not in _CACHE:
        _CACHE["nc"] = _build_module()
        _CACHE["consts"] = _build_consts()
    return _CACHE["nc"], _CACHE["consts"]


def kernel(x: np.ndarray) -> np.ndarray:
    x = np.ascontiguousarray(np.asarray(x, dtype=np.float32))
    assert x.shape == (B, C, H, W), x.shape

    nc, cst = _get_module()
    in_maps = [{"x": x[b], "cst": cst} for b in range(N_CORES)]
    res = run_bass_kernel_spmd(nc, in_maps, core_ids=list(range(N_CORES)))
    out = np.stack(
        [np.asarray(res.results[b]["out"]) for b in range(N_CORES)], axis=0)
    return out.astype(np.float32)


# revision 14
# speedup vs baseline: 1.0169x; 1.0169x over previous
"""Trainium2 Bass kernel: block 8x8 2D-DCT + channel-pack + 8x nearest upsample.

Computes, for input x (8, 3, 256, 256) f32:
  out[b, 64c+8a+d, 8i+r, 8j+q] = sum_{m,n} D[a,m] x[b,c,8i+m,8j+n] D[d,n]
i.e. the reference nn_DCT2D: per-8x8-block orthonormal DCT-II, 64 coeffs packed
into channels, then 8x8 nearest-neighbor upsample back to (256, 256).

Strategy (pure data-parallel over batch, one core per batch element):
  - Consts: one [128, 640] f32 tensor (M''[kt=0] | M''[kt=1] | R2) loaded in
    a single 128x2.5KB-descriptor HWDGE DMA at t=0 (fast startup).
  - Step 1 (TensorE, f32): A2 = X^T @ M'', the row-DCT over H, where M'' is
    the block-diagonal DCT factor with columns permuted to c'' = ie*128+8*ip+a
    (i = 2*ip + ie). Output A2[kh] [128 x 256] for the two n_img halves.
  - Step 2 (TensorE, f32): per (c, ie, kh) ONE matmul
    ps2[:, kh*128:+128] = A2[kh][:, ie-half]^T @ R2 with R2[k', 16d+jj] =
    D[d, k'%8]*[k'//8 == jj]: all 8 output-channel phases d in one
    [contract 128 x free 128] matmul (no upsample in the matmul).
  - Copy stage A (DVE/ACT): per (c, ie, d) cast ps2[:, 16d:16d+16] f32 ->
    f16 PAIRS qq[p, kh, jj, 2] (64 el) -- each coefficient duplicated twice
    in adjacent f16 slots.
  - Copy stage B (DVE/ACT, 4:3 split favoring DVE): per (c, d, ie, kh)
    broadcast-copy qq bitcast as f32 (one 4-byte element = 2 identical f16)
    to [128, 8r, 16jj, 4qp]: the 8x H-replication and the remaining 4x of
    the W-replication at HALF the element count of a plain f16 fanout.
    Lands in per-(c,d) f16 tiles o4 [128 x 4096], partition p = 8*ip+a,
    free f = ie*2048 + r*256 + kh*128 + jj*8 + q.
  - DMA out (both HWDGE rings, alternating): one 1 MB f16 DMA per (c, d)
    with 8 KB descriptors: partition (ip, a) -> channel 64c+8a+d rows
    [16ip, 16ip+16), contiguous in HBM. The last two tiles are split into
    ie-halves across both rings to shorten the drain tail. Host upcasts
    f16 -> f32 after the gather; f16 storage error (~3e-4 of scale) is far
    inside the 2e-2 gate and halves the dominant HBM write traffic.

Matmuls and PSUM accumulation in f32; only the HBM output tensor is f16.
"""

import numpy as np

import concourse.bacc as bacc
import concourse.mybir as mybir
from concourse.tile import TileContext
from concourse.bass_utils import run_bass_kernel_spmd

N_CORES = 8
B, C, H, W = 8, 3, 256, 256
BS = 8          # DCT block size
F32 = mybir.dt.float32
F16 = mybir.dt.float16


def _dct_matrix() -> np.ndarray:
    n = np.arange(BS, dtype=np.float64)
    k = n[:, None]
    D = np.cos(np.pi * (2.0 * n[None, :] + 1.0) * k / (2.0 * BS))
    scale = np.full((BS,), np.sqrt(2.0 / BS))
    scale[0] = np.sqrt(1.0 / BS)
    return (D * scale[:, None]).astype(np.float32)


def _build_consts() -> np.ndarray:
    D = _dct_matrix()
    # M'' [256, 256]: col c'' = ie*128 + 8*ip + a maps to DCT row
    # 32a + i with i = 2*ip + ie:  M''[k, c''] = D[a, k%8] iff k//8 == i.
    # (ip-major partition order so the output DMA's outer dst dim has 16
    # entries -> descriptors spread over all 16 SDMA engines.)
    Mpp = np.zeros((256, 256), np.float32)
    for k in range(256):
        i = k // 8
        for a in range(8):
            Mpp[k, (i % 2) * 128 + 8 * (i // 2) + a] = D[a, k % 8]

    # R2 [128, 128]: second DCT factor, all d packed in the free dim.
    # R2[k', 16d+jj] = D[d, k'%8] iff k'//8 == jj.
    R2 = np.zeros((128, 128), np.float32)
    for kp in range(128):
        jj = kp // 8
        for d in range(8):
            R2[kp, 16 * d + jj] = D[d, kp % 8]

    # single [128, 640] partition-major tensor: 2.5KB per descriptor.
    return np.concatenate([Mpp[:128], Mpp[128:], R2], axis=1)


def _build_module():
    nc = bacc.Bacc("TRN2", target_bir_lowering=False, debug=False,
                   enable_asserts=False)

    x_t = nc.dram_tensor("x", [C, H, W], F32, kind="ExternalInput")
    c_t = nc.dram_tensor("cst", [128, 640], F32, kind="ExternalInput")
    out_t = nc.dram_tensor("out", [C * 64, H, W], F16, kind="ExternalOutput")
    # view with channel split into (c, a, d) for the output APs
    out_r = out_t.rearrange("(c a d) h w -> c a d h w", c=C, a=8, d=8)

    with TileContext(nc) as tc:
        with (
            tc.tile_pool(name="consts", bufs=1) as cpool,
            tc.tile_pool(name="xp", bufs=6) as xpool,
            tc.tile_pool(name="atp", bufs=4) as atpool,
            tc.tile_pool(name="qqp", bufs=6) as qqpool,
            tc.tile_pool(name="outp", bufs=10) as opool,
            tc.tile_pool(name="psa", bufs=2, space="PSUM") as psa_pool,
            tc.tile_pool(name="ps2", bufs=4, space="PSUM") as ps2_pool,
        ):
            # consts on the Act HWDGE ring, x[c=0] split across both HWDGE
            # rings (all idle at t=0); later x images on the Pool SWDGE ring
            # to keep the HWDGE rings clear for output.
            ct = cpool.tile_from(c_t[:, :], name="consts",
                                 forced_dma_engine=mybir.EngineType.SP)
            m_ap = [ct[:, 0:256], ct[:, 256:512]]
            r2_ap = ct[:, 512:640]

            xt = []
            for c in range(C):
                for kt in range(2):
                    tile = xpool.tile([128, 256], F32, tag="x")
                    if c == 0:
                        eng = nc.scalar if kt == 0 else nc.sync
                    else:
                        eng = nc.gpsimd
                    eng.dma_start(out=tile[:, :],
                                  in_=x_t[c, kt * 128:(kt + 1) * 128, :])
                    xt.append(tile)

            qi = 0   # output DMA queue round-robin
            cp = 0   # copy engine round-robin
            for c in range(C):
                # step 1: A2[kh] [n_img-half, c''=(ie, ip, a)]
                at = []
                for kh in range(2):
                    ps_a = psa_pool.tile([128, 256], F32, tag="psa")
                    for kt in range(2):
                        nc.tensor.matmul(
                            ps_a[:, :],
                            lhsT=xt[2 * c + kt][:, kh * 128:(kh + 1) * 128],
                            rhs=m_ap[kt],
                            start=(kt == 0), stop=(kt == 1),
                        )
                    a_sb = atpool.tile([128, 256], F32, tag="at")
                    if kh == 0:
                        nc.vector.tensor_copy(out=a_sb[:, :], in_=ps_a[:, :])
                    else:
                        nc.scalar.copy(out=a_sb[:, :], in_=ps_a[:, :])
                    at.append(a_sb)

                # step 2: per (ie, kh) one matmul computing all d at once:
                # ps2[ie][p=(ip,a), kh*128 + 16d + jj] = coeff y[c,(a,d),i,j]
                # with i = 2ip+ie, j = 16kh+jj.
                ps2 = []
                for ie in range(2):
                    t = ps2_pool.tile([128, 256], F32, tag="ps2")
                    for kh in range(2):
                        nc.tensor.matmul(
                            t[:, kh * 128:(kh + 1) * 128],
                            lhsT=at[kh][:, ie * 128:(ie + 1) * 128],
                            rhs=r2_ap,
                            start=True, stop=True,
                        )
                    ps2.append(t)
                ps2v = [p.rearrange("p (kh f) -> p kh f", kh=2) for p in ps2]

                # copies + 1 MB DMA per (c, d)
                for d in range(8):
                    o4 = opool.tile([128, 4096], F16, tag="o4")
                    ov32 = o4.bitcast(F32).rearrange(
                        "p (ie r kh jj qp) -> p ie kh r jj qp",
                        ie=2, r=8, kh=2, jj=16, qp=4)
                    for ie in range(2):
                        # stage A: f16 pairs qq[p, kh, jj, 2]
                        qq = qqpool.tile([128, 64], F16, tag="qq")
                        src = ps2v[ie][:, :, 16 * d:16 * d + 16]
                        src = src[:, :, :, None].to_broadcast([128, 2, 16, 2])
                        dst = qq.rearrange("p (kh jj pr) -> p kh jj pr",
                                           kh=2, jj=16, pr=2)
                        if cp % 2 == 0:
                            nc.vector.tensor_copy(out=dst, in_=src)
                        else:
                            nc.scalar.copy(out=dst, in_=src)
                        cp += 1
                        # stage B: pair-granular fanout, f32 bitcast
                        qq32 = qq.bitcast(F32)
                        for kh in range(2):
                            src32 = qq32[:, kh * 16:(kh + 1) * 16]
                            src32 = src32[:, None, :, None].to_broadcast(
                                [128, 8, 16, 4])
                            if cp % 2 == 0:
                                nc.vector.tensor_copy(out=ov32[:, ie, kh],
                                                      in_=src32)
                            else:
                                nc.scalar.copy(out=ov32[:, ie, kh], in_=src32)
                            cp += 1

                    # one 1 MB f16 DMA: partition (ip, a) -> rows
                    # [16ip, 16ip+16) of channel (c, a, d); 8 KB descriptors.
                    # First tiles (pipeline ramp) and last tiles (drain):
                    # split into ie-halves across both rings (4 KB
                    # descriptors) so the stream starts earlier / ends
                    # together.
                    if (c == 0 and d < 4) or (c == C - 1 and d >= 6):
                        dsth = out_r[c, :, d].rearrange(
                            "a (ip ie hh) w -> ie ip a (hh w)", ip=16, ie=2,
                            hh=8)
                        for ie in range(2):
                            eng = nc.sync if (qi + ie) % 2 == 0 else nc.scalar
                            eng.dma_start(out=dsth[ie],
                                          in_=o4[:, ie * 2048:(ie + 1) * 2048])
                        qi += 1
                    else:
                        dst = out_r[c, :, d].rearrange(
                            "a (ip hh) w -> ip a (hh w)", hh=16)
                        eng = nc.scalar if qi % 2 == 1 else nc.sync
                        qi += 1
                        eng.dma_start(out=dst, in_=o4[:, :])

    nc.compile()
    return nc


_CACHE: dict = {}


def _get_module():
    if "nc" not in _CACHE:
        _CACHE["nc"] = _build_module()
        _CACHE["consts"] = _build_consts()
    return _CACHE["nc"], _CACHE["consts"]


def kernel(x: np.ndarray) -> np.ndarray:
    x = np.ascontiguousarray(np.asarray(x, dtype=np.float32))
    assert x.shape == (B, C, H, W), x.shape

    nc, cst = _get_module()
    in_maps = [{"x": x[b], "cst": cst} for b in range(N_CORES)]
    res = run_bass_kernel_spmd(nc, in_maps, core_ids=list(range(N_CORES)))
    out = np.stack(
        [np.asarray(res.results[b]["out"]) for b in range(N_CORES)], axis=0)
    return out.astype(np.float32)
